# revision 17
# baseline (speedup 1.0000x reference)
"""clDice loss kernel for Trainium2 (8 NeuronCores, batch-data-parallel).

Each core processes one [1024,1024] image pair (sigmoid + 10-iter soft
skeletonization + partial sums); the host combines per-core partial sums
in float64 and applies the dice/clDice formulas.

Layout: image row r lives at (partition p = r//8, block n = r%8), i.e.
an SBUF tile [128, 8, 1024].  3-point min/max pools are built from two
shifted tensor_tensor ops; vertical (cross-row) shifts that cross
partitions go through small SBUF->SBUF DMA halo copies with a
replicated-edge convention so every compute op spans all 128 partitions.
"""

import numpy as np

P, NB, W = 128, 8, 1024
N_ITER = 10
N_CORES = 8
SMOOTH = 1.0
EPS = 1e-7
ALPHA = 0.5

# engine knobs: 'v' = vector (DVE), 'g' = gpsimd
ENG_HPAIR_MIN = 'v'   # hpool pair op (erode path)
ENG_HCOMB_MIN = 'v'   # hpool combine op (erode path)
ENG_HPAIR_MAX = 'v'   # hpool pair op (dilate path)
ENG_HCOMB_MAX = 'v'   # hpool combine op (dilate path)
ENG_MULT = 'v'        # skel-update mul
ENG_ADD = 'g'         # skel-update add

_CACHE = {}


def _build_nc():
    import concourse.bacc as bacc
    import concourse.mybir as mybir
    import concourse.tile as tile

    DT = mybir.dt.bfloat16
    F32 = mybir.dt.float32
    I32 = mybir.dt.int32
    MIN = mybir.AluOpType.min
    MAX = mybir.AluOpType.max
    SUB = mybir.AluOpType.subtract
    ADD = mybir.AluOpType.add
    MULT = mybir.AluOpType.mult
    AF = mybir.ActivationFunctionType
    XY = mybir.AxisListType.XY

    nc = bacc.Bacc("TRN2", target_bir_lowering=False, debug=False,
                   num_devices=N_CORES)
    lg = nc.dram_tensor("logits", [P * NB, W], F32, kind="ExternalInput")
    tg = nc.dram_tensor("target", [P * NB, W], I32, kind="ExternalInput")
    pr = nc.dram_tensor("partials", [P, 8], F32, kind="ExternalOutput")

    lgv = lg.ap().rearrange("(p n) m -> p n m", n=NB)
    tgv = tg.ap().rearrange("(p n) m -> p n m", n=NB)

    with tile.TileContext(nc) as tc:
        from contextlib import ExitStack
        ctx = ExitStack()
        with ctx:
            im_pool = ctx.enter_context(tc.tile_pool(name="im", bufs=2))
            sk_pool = ctx.enter_context(tc.tile_pool(name="sk", bufs=3))
            s_pool = ctx.enter_context(tc.tile_pool(name="s", bufs=4))
            halo_pool = ctx.enter_context(tc.tile_pool(name="halo", bufs=6))
            stage_pool = ctx.enter_context(tc.tile_pool(name="stage", bufs=2))
            p_pool = ctx.enter_context(tc.tile_pool(name="part", bufs=1))

            partials = p_pool.tile([P, 8], F32, tag="pp", name="pp")
            nc.vector.memset(partials[:, :], 0.0)

            def eng(which):
                return nc.gpsimd if which == 'g' else nc.vector

            def img_tile(pool, tag):
                return pool.tile([P, NB, W], DT, tag=tag, name=tag)

            def pool3_h(dst, src, op, e_pair='v', e_comb='v', split_n0=False):
                # horizontal 3-pool along the W axis (shrink-at-edge)
                b = img_tile(s_pool, "s")
                eng(e_pair).tensor_tensor(
                    b[:, :, 0:W - 1], src[:, :, 0:W - 1], src[:, :, 1:W], op=op)
                nc.scalar.copy(dst[:, :, 0:1], b[:, :, 0:1])
                nc.scalar.copy(dst[:, :, W - 1:W], b[:, :, W - 2:W - 1])
                if split_n0:
                    # emit block-0 rows first so a following vertical pool
                    # can start its halo DMA early
                    eng(e_comb).tensor_tensor(
                        dst[:, 0, 1:W - 1], b[:, 0, 0:W - 2], src[:, 0, 2:W], op=op)
                    eng(e_comb).tensor_tensor(
                        dst[:, NB - 1, 1:W - 1], b[:, NB - 1, 0:W - 2],
                        src[:, NB - 1, 2:W], op=op)
                    eng(e_comb).tensor_tensor(
                        dst[:, 1:NB - 1, 1:W - 1], b[:, 1:NB - 1, 0:W - 2],
                        src[:, 1:NB - 1, 2:W], op=op)
                else:
                    eng(e_comb).tensor_tensor(
                        dst[:, :, 1:W - 1], b[:, :, 0:W - 2], src[:, :, 2:W], op=op)

            def pool3_v(dst, src, op):
                # vertical 3-pool across rows r = 8p+n (shrink-at-edge).
                # Both halo DMAs depend only on src, so they fire at pool
                # start and stay off the DVE critical path.
                a = img_tile(s_pool, "s")
                xd = halo_pool.tile([P, W], DT, tag="h", name="xd")
                # xd[p] = row below partition p's last row (replicate at bottom)
                nc.sync.dma_start(xd[0:P - 1, :], src[1:P, 0, :])
                nc.sync.dma_start(xd[P - 1:P, :], src[P - 1:P, NB - 1, :])
                xu = halo_pool.tile([P, W], DT, tag="h", name="xu")
                # xu[p] = row above partition p's first row (replicate at top
                # with row 0 itself so the pair below stays exact)
                nc.scalar.dma_start(xu[1:P, :], src[0:P - 1, NB - 1, :])
                nc.scalar.dma_start(xu[0:1, :], src[0:1, 0, :])
                nc.vector.tensor_tensor(
                    a[:, 0:NB - 1, :], src[:, 0:NB - 1, :], src[:, 1:NB, :], op=op)
                nc.vector.tensor_tensor(
                    a[:, NB - 1, :], src[:, NB - 1, :], xd[:, :], op=op)
                au = halo_pool.tile([P, W], DT, tag="h", name="au")
                # au[p] = pair of rows (8p-1, 8p); at top: row 0 itself
                nc.vector.tensor_tensor(
                    au[:, :], xu[:, :], src[:, 0, :], op=op)
                nc.vector.tensor_tensor(
                    dst[:, 1:NB - 1, :], a[:, 0:NB - 2, :], src[:, 2:NB, :], op=op)
                nc.vector.tensor_tensor(
                    dst[:, NB - 1, :], a[:, NB - 2, :], xd[:, :], op=op)
                nc.vector.tensor_tensor(
                    dst[:, 0, :], au[:, :], src[:, 1, :], op=op)

            def erode(dst, src):
                vmin = img_tile(s_pool, "s")
                pool3_v(vmin, src, MIN)
                hmin = img_tile(s_pool, "s")
                pool3_h(hmin, src, MIN, ENG_HPAIR_MIN, ENG_HCOMB_MIN)
                # split the combine so the consumer vpool's halo DMAs (which
                # need blocks 0 and 7 of dst) can start before the bulk is done
                nc.vector.tensor_tensor(
                    dst[:, 0, :], vmin[:, 0, :], hmin[:, 0, :], op=MIN)
                nc.vector.tensor_tensor(
                    dst[:, NB - 1, :], vmin[:, NB - 1, :], hmin[:, NB - 1, :], op=MIN)
                nc.vector.tensor_tensor(
                    dst[:, 1:NB - 1, :], vmin[:, 1:NB - 1, :],
                    hmin[:, 1:NB - 1, :], op=MIN)

            def dilate(dst, src):
                hm = img_tile(s_pool, "s")
                pool3_h(hm, src, MAX, ENG_HPAIR_MAX, ENG_HCOMB_MAX, split_n0=True)
                pool3_v(dst, hm, MAX)

            def stream(im0, accum_col):
                # soft_skel with one erode per iteration; returns final skel
                e = img_tile(im_pool, "im")
                erode(e, im0)
                opn = img_tile(s_pool, "s")
                dilate(opn, e)
                t1 = img_tile(s_pool, "s")
                nc.vector.tensor_tensor(t1[:, :, :], im0[:, :, :], opn[:, :, :], op=SUB)
                sk = img_tile(sk_pool, "sk")
                nc.scalar.activation(sk[:, :, :], t1[:, :, :], AF.Relu)
                im = e
                for j in range(N_ITER):
                    # r depends only on last iteration's sk — emit first so
                    # ACT computes it during the erode/dilate phase
                    r = img_tile(s_pool, "s")
                    nc.scalar.activation(r[:, :, :], sk[:, :, :], AF.Relu,
                                         bias=1.0, scale=-1.0)
                    e = img_tile(im_pool, "im")
                    erode(e, im)
                    opn = img_tile(s_pool, "s")
                    dilate(opn, e)
                    t1 = img_tile(s_pool, "s")
                    nc.vector.tensor_tensor(
                        t1[:, :, :], im[:, :, :], opn[:, :, :], op=SUB)
                    # relu(t1)*r == relu(t1*r) since r >= 0: skip the ACT relu
                    # and clamp with a cheap 4x-mode tensor_scalar instead
                    q = img_tile(s_pool, "s")
                    nc.vector.tensor_tensor(
                        q[:, :, :], t1[:, :, :], r[:, :, :], op=MULT)
                    prod = img_tile(s_pool, "s")
                    nc.vector.tensor_scalar_max(prod[:, :, :], q[:, :, :], 0.0)
                    sk_new = img_tile(sk_pool, "sk")
                    if j < N_ITER - 1:
                        nc.vector.tensor_tensor(
                            sk_new[:, :, :], sk[:, :, :], prod[:, :, :], op=ADD)
                    else:
                        nc.vector.scalar_tensor_tensor(
                            sk_new[:, :, :], prod[:, :, :], 0.0, sk[:, :, :],
                            op0=ADD, op1=ADD,
                            accum_out=partials[:, accum_col:accum_col + 1])
                    sk = sk_new
                    im = e
                return sk

            # --- init: probs (sigmoid) + dice partial sums ---
            p0 = img_tile(im_pool, "im")
            for h in range(2):
                st = stage_pool.tile([P, NB // 2, W], F32, tag="st", name="st")
                nc.gpsimd.dma_start(st[:, :, :], lgv[:, 4 * h:4 * h + 4, :])
                nc.scalar.activation(p0[:, 4 * h:4 * h + 4, :], st[:, :, :],
                                     AF.Sigmoid,
                                     accum_out=partials[:, h:h + 1])
            tb = img_tile(s_pool, "s")
            for h in range(2):
                st = stage_pool.tile([P, NB // 2, W], I32, tag="st", name="st")
                nc.gpsimd.dma_start(st[:, :, :], tgv[:, 4 * h:4 * h + 4, :])
                nc.vector.tensor_copy(tb[:, 4 * h:4 * h + 4, :], st[:, :, :])
            nc.vector.tensor_reduce(partials[:, 2:3], tb[:, :, :], axis=XY, op=ADD)
            pt_scr = img_tile(s_pool, "s")
            nc.vector.scalar_tensor_tensor(
                pt_scr[:, :, :], p0[:, :, :], 0.0, tb[:, :, :],
                op0=ADD, op1=MULT, accum_out=partials[:, 3:4])

            # prefetch the target reload into stage tiles; the DMAs fire as
            # soon as the init casts release the slots, long before needed
            re_st = []
            for h in range(2):
                st = stage_pool.tile([P, NB // 2, W], I32, tag="st", name="st")
                nc.gpsimd.dma_start(st[:, :, :], tgv[:, 4 * h:4 * h + 4, :])
                re_st.append(st)

            # --- probs stream ---
            sk_o = stream(p0, accum_col=4)

            # --- target stream (cast prefetched reload) ---
            t0 = img_tile(im_pool, "im")
            for h in range(2):
                nc.vector.tensor_copy(t0[:, 4 * h:4 * h + 4, :], re_st[h][:, :, :])
            sk_t = stream(t0, accum_col=5)

            sost_scr = img_tile(s_pool, "s")
            nc.vector.scalar_tensor_tensor(
                sost_scr[:, :, :], sk_o[:, :, :], 0.0, sk_t[:, :, :],
                op0=ADD, op1=MULT, accum_out=partials[:, 6:7])

            nc.sync.dma_start(pr.ap(), partials[:, :])

    nc.compile()
    return nc


def get_nc():
    if "nc" not in _CACHE:
        _CACHE["nc"] = _build_nc()
    return _CACHE["nc"]


def combine_partials(parts):
    """parts: [n_cores, 128, 8] float array -> scalar loss (float32)."""
    parts = np.asarray(parts, dtype=np.float64)
    sum_p = parts[:, :, 0].sum() + parts[:, :, 1].sum()
    sum_t = parts[:, :, 2].sum()
    sum_pt = parts[:, :, 3].sum()
    so = parts[:, :, 4].sum()
    st = parts[:, :, 5].sum()
    sost = parts[:, :, 6].sum()

    inter, card = sum_pt, sum_p + sum_t
    score = (2.0 * inter + SMOOTH) / max(card + SMOOTH, EPS)
    dice = (1.0 - score) * (1.0 if sum_t > 0 else 0.0)

    tprec = (sost + SMOOTH) / (so + SMOOTH)
    tsens = (sost + SMOOTH) / (st + SMOOTH)
    cl = 2.0 * tprec * tsens / (tprec + tsens)
    cld = (1.0 - cl) * (1.0 if st > 0 else 0.0)
    return np.float32((1.0 - ALPHA) * dice + ALPHA * cld)


def run_partials(output, target, trace=False):
    from concourse import bass_utils
    nc = get_nc()
    in_maps = []
    for c in range(N_CORES):
        in_maps.append({
            "logits": np.ascontiguousarray(output[c, 0], dtype=np.float32),
            "target": np.ascontiguousarray(target[c, 0], dtype=np.int32),
        })
    res = bass_utils.run_bass_kernel_spmd(
        nc, in_maps, core_ids=list(range(N_CORES)), trace=trace)
    parts = np.stack([res.results[c]["partials"] for c in range(N_CORES)])
    return parts, res


def kernel(output, target):
    parts, _ = run_partials(output, target)
    return combine_partials(parts)


# revision 18
# speedup vs baseline: 1.0162x; 1.0162x over previous
"""clDice loss kernel for Trainium2 (8 NeuronCores, batch-data-parallel).

Each core processes one [1024,1024] image pair (sigmoid + 10-iter soft
skeletonization + partial sums); the host combines per-core partial sums
in float64 and applies the dice/clDice formulas.

Layout: image row r lives at (partition p = r//8, block n = r%8), i.e.
an SBUF tile [128, 8, 1024].  3-point min/max pools are built from two
shifted tensor_tensor ops; vertical (cross-row) shifts that cross
partitions go through small SBUF->SBUF DMA halo copies with a
replicated-edge convention so every compute op spans all 128 partitions.
"""

import numpy as np

P, NB, W = 128, 8, 1024
N_ITER = 10
N_CORES = 8
SMOOTH = 1.0
EPS = 1e-7
ALPHA = 0.5

# engine knobs: 'v' = vector (DVE), 'g' = gpsimd
ENG_HPAIR_MIN = 'v'   # hpool pair op (erode path)
ENG_HCOMB_MIN = 'v'   # hpool combine op (erode path)
ENG_HPAIR_MAX = 'v'   # hpool pair op (dilate path)
ENG_HCOMB_MAX = 'v'   # hpool combine op (dilate path)
ENG_MULT = 'v'        # skel-update mul
ENG_ADD = 'g'         # skel-update add

_CACHE = {}


def _build_nc():
    import concourse.bacc as bacc
    import concourse.mybir as mybir
    import concourse.tile as tile

    DT = mybir.dt.bfloat16
    F32 = mybir.dt.float32
    I32 = mybir.dt.int32
    MIN = mybir.AluOpType.min
    MAX = mybir.AluOpType.max
    SUB = mybir.AluOpType.subtract
    ADD = mybir.AluOpType.add
    MULT = mybir.AluOpType.mult
    AF = mybir.ActivationFunctionType
    XY = mybir.AxisListType.XY

    nc = bacc.Bacc("TRN2", target_bir_lowering=False, debug=False,
                   num_devices=N_CORES)
    lg = nc.dram_tensor("logits", [P * NB, W], F32, kind="ExternalInput")
    tg = nc.dram_tensor("target", [P * NB, W], I32, kind="ExternalInput")
    pr = nc.dram_tensor("partials", [P, 8], F32, kind="ExternalOutput")

    lgv = lg.ap().rearrange("(p n) m -> p n m", n=NB)
    tgv = tg.ap().rearrange("(p n) m -> p n m", n=NB)

    with tile.TileContext(nc) as tc:
        from contextlib import ExitStack
        ctx = ExitStack()
        with ctx:
            im_pool = ctx.enter_context(tc.tile_pool(name="im", bufs=2))
            sk_pool = ctx.enter_context(tc.tile_pool(name="sk", bufs=3))
            s_pool = ctx.enter_context(tc.tile_pool(name="s", bufs=4))
            halo_pool = ctx.enter_context(tc.tile_pool(name="halo", bufs=6))
            stage_pool = ctx.enter_context(tc.tile_pool(name="stage", bufs=2))
            p_pool = ctx.enter_context(tc.tile_pool(name="part", bufs=1))

            partials = p_pool.tile([P, 8], F32, tag="pp", name="pp")
            nc.vector.memset(partials[:, :], 0.0)

            def eng(which):
                return nc.gpsimd if which == 'g' else nc.vector

            def img_tile(pool, tag):
                return pool.tile([P, NB, W], DT, tag=tag, name=tag)

            def pool3_h(dst, src, op, e_pair='v', e_comb='v', split_n0=False):
                # horizontal 3-pool along the W axis (shrink-at-edge)
                b = img_tile(s_pool, "s")
                eng(e_pair).tensor_tensor(
                    b[:, :, 0:W - 1], src[:, :, 0:W - 1], src[:, :, 1:W], op=op)
                nc.scalar.copy(dst[:, :, 0:1], b[:, :, 0:1])
                nc.scalar.copy(dst[:, :, W - 1:W], b[:, :, W - 2:W - 1])
                if split_n0:
                    # emit block-0 rows first so a following vertical pool
                    # can start its halo DMA early
                    eng(e_comb).tensor_tensor(
                        dst[:, 0, 1:W - 1], b[:, 0, 0:W - 2], src[:, 0, 2:W], op=op)
                    eng(e_comb).tensor_tensor(
                        dst[:, NB - 1, 1:W - 1], b[:, NB - 1, 0:W - 2],
                        src[:, NB - 1, 2:W], op=op)
                    eng(e_comb).tensor_tensor(
                        dst[:, 1:NB - 1, 1:W - 1], b[:, 1:NB - 1, 0:W - 2],
                        src[:, 1:NB - 1, 2:W], op=op)
                else:
                    eng(e_comb).tensor_tensor(
                        dst[:, :, 1:W - 1], b[:, :, 0:W - 2], src[:, :, 2:W], op=op)

            def pool3_v(dst, src, op):
                # vertical 3-pool across rows r = 8p+n (shrink-at-edge).
                # Both halo DMAs depend only on src, so they fire at pool
                # start and stay off the DVE critical path.
                a = img_tile(s_pool, "s")
                xd = halo_pool.tile([P, W], DT, tag="h", name="xd")
                # xd[p] = row below partition p's last row (replicate at
                # bottom).  Single-partition DMAs lower to 16x128B
                # descriptors (~10us), so write a wide replicate band first
                # and let the main shift overwrite all but the last row.
                nc.sync.dma_start(xd[P - 32:P, :], src[P - 32:P, NB - 1, :])
                nc.sync.dma_start(xd[0:P - 1, :], src[1:P, 0, :])
                xu = halo_pool.tile([P, W], DT, tag="h", name="xu")
                # xu[p] = row above partition p's first row (replicate at top
                # with row 0 itself so the pair below stays exact)
                nc.scalar.dma_start(xu[0:32, :], src[0:32, 0, :])
                nc.scalar.dma_start(xu[1:P, :], src[0:P - 1, NB - 1, :])
                nc.vector.tensor_tensor(
                    a[:, 0:NB - 1, :], src[:, 0:NB - 1, :], src[:, 1:NB, :], op=op)
                nc.vector.tensor_tensor(
                    a[:, NB - 1, :], src[:, NB - 1, :], xd[:, :], op=op)
                au = halo_pool.tile([P, W], DT, tag="h", name="au")
                # au[p] = pair of rows (8p-1, 8p); at top: row 0 itself
                nc.vector.tensor_tensor(
                    au[:, :], xu[:, :], src[:, 0, :], op=op)
                nc.vector.tensor_tensor(
                    dst[:, 1:NB - 1, :], a[:, 0:NB - 2, :], src[:, 2:NB, :], op=op)
                nc.vector.tensor_tensor(
                    dst[:, NB - 1, :], a[:, NB - 2, :], xd[:, :], op=op)
                nc.vector.tensor_tensor(
                    dst[:, 0, :], au[:, :], src[:, 1, :], op=op)

            def erode(dst, src):
                vmin = img_tile(s_pool, "s")
                pool3_v(vmin, src, MIN)
                hmin = img_tile(s_pool, "s")
                pool3_h(hmin, src, MIN, ENG_HPAIR_MIN, ENG_HCOMB_MIN)
                # split the combine so the consumer vpool's halo DMAs (which
                # need blocks 0 and 7 of dst) can start before the bulk is done
                nc.vector.tensor_tensor(
                    dst[:, 0, :], vmin[:, 0, :], hmin[:, 0, :], op=MIN)
                nc.vector.tensor_tensor(
                    dst[:, NB - 1, :], vmin[:, NB - 1, :], hmin[:, NB - 1, :], op=MIN)
                nc.vector.tensor_tensor(
                    dst[:, 1:NB - 1, :], vmin[:, 1:NB - 1, :],
                    hmin[:, 1:NB - 1, :], op=MIN)

            def dilate(dst, src):
                hm = img_tile(s_pool, "s")
                pool3_h(hm, src, MAX, ENG_HPAIR_MAX, ENG_HCOMB_MAX, split_n0=True)
                pool3_v(dst, hm, MAX)

            def stream(im0, accum_col):
                # soft_skel with one erode per iteration; returns final skel
                e = img_tile(im_pool, "im")
                erode(e, im0)
                opn = img_tile(s_pool, "s")
                dilate(opn, e)
                t1 = img_tile(s_pool, "s")
                nc.vector.tensor_tensor(t1[:, :, :], im0[:, :, :], opn[:, :, :], op=SUB)
                sk = img_tile(sk_pool, "sk")
                nc.scalar.activation(sk[:, :, :], t1[:, :, :], AF.Relu)
                im = e
                for j in range(N_ITER):
                    # r depends only on last iteration's sk — emit first so
                    # ACT computes it during the erode/dilate phase
                    r = img_tile(s_pool, "s")
                    nc.scalar.activation(r[:, :, :], sk[:, :, :], AF.Relu,
                                         bias=1.0, scale=-1.0)
                    e = img_tile(im_pool, "im")
                    erode(e, im)
                    opn = img_tile(s_pool, "s")
                    dilate(opn, e)
                    t1 = img_tile(s_pool, "s")
                    nc.vector.tensor_tensor(
                        t1[:, :, :], im[:, :, :], opn[:, :, :], op=SUB)
                    # relu(t1)*r == relu(t1*r) since r >= 0: skip the ACT relu
                    # and clamp with a cheap 4x-mode tensor_scalar instead
                    q = img_tile(s_pool, "s")
                    nc.vector.tensor_tensor(
                        q[:, :, :], t1[:, :, :], r[:, :, :], op=MULT)
                    prod = img_tile(s_pool, "s")
                    nc.vector.tensor_scalar_max(prod[:, :, :], q[:, :, :], 0.0)
                    sk_new = img_tile(sk_pool, "sk")
                    if j < N_ITER - 1:
                        nc.vector.tensor_tensor(
                            sk_new[:, :, :], sk[:, :, :], prod[:, :, :], op=ADD)
                    else:
                        nc.vector.scalar_tensor_tensor(
                            sk_new[:, :, :], prod[:, :, :], 0.0, sk[:, :, :],
                            op0=ADD, op1=ADD,
                            accum_out=partials[:, accum_col:accum_col + 1])
                    sk = sk_new
                    im = e
                return sk

            # --- init: probs (sigmoid) + dice partial sums ---
            p0 = img_tile(im_pool, "im")
            for h in range(2):
                st = stage_pool.tile([P, NB // 2, W], F32, tag="st", name="st")
                nc.gpsimd.dma_start(st[:, :, :], lgv[:, 4 * h:4 * h + 4, :])
                nc.scalar.activation(p0[:, 4 * h:4 * h + 4, :], st[:, :, :],
                                     AF.Sigmoid,
                                     accum_out=partials[:, h:h + 1])
            tb = img_tile(s_pool, "s")
            for h in range(2):
                st = stage_pool.tile([P, NB // 2, W], I32, tag="st", name="st")
                nc.gpsimd.dma_start(st[:, :, :], tgv[:, 4 * h:4 * h + 4, :])
                nc.vector.tensor_copy(tb[:, 4 * h:4 * h + 4, :], st[:, :, :])
            nc.vector.tensor_reduce(partials[:, 2:3], tb[:, :, :], axis=XY, op=ADD)
            pt_scr = img_tile(s_pool, "s")
            nc.vector.scalar_tensor_tensor(
                pt_scr[:, :, :], p0[:, :, :], 0.0, tb[:, :, :],
                op0=ADD, op1=MULT, accum_out=partials[:, 3:4])

            # prefetch the target reload into stage tiles; the DMAs fire as
            # soon as the init casts release the slots, long before needed
            re_st = []
            for h in range(2):
                st = stage_pool.tile([P, NB // 2, W], I32, tag="st", name="st")
                nc.gpsimd.dma_start(st[:, :, :], tgv[:, 4 * h:4 * h + 4, :])
                re_st.append(st)

            # --- probs stream ---
            sk_o = stream(p0, accum_col=4)

            # --- target stream (cast prefetched reload) ---
            t0 = img_tile(im_pool, "im")
            for h in range(2):
                nc.vector.tensor_copy(t0[:, 4 * h:4 * h + 4, :], re_st[h][:, :, :])
            sk_t = stream(t0, accum_col=5)

            sost_scr = img_tile(s_pool, "s")
            nc.vector.scalar_tensor_tensor(
                sost_scr[:, :, :], sk_o[:, :, :], 0.0, sk_t[:, :, :],
                op0=ADD, op1=MULT, accum_out=partials[:, 6:7])

            nc.sync.dma_start(pr.ap(), partials[:, :])

    nc.compile()
    return nc


def get_nc():
    if "nc" not in _CACHE:
        _CACHE["nc"] = _build_nc()
    return _CACHE["nc"]


def combine_partials(parts):
    """parts: [n_cores, 128, 8] float array -> scalar loss (float32)."""
    parts = np.asarray(parts, dtype=np.float64)
    sum_p = parts[:, :, 0].sum() + parts[:, :, 1].sum()
    sum_t = parts[:, :, 2].sum()
    sum_pt = parts[:, :, 3].sum()
    so = parts[:, :, 4].sum()
    st = parts[:, :, 5].sum()
    sost = parts[:, :, 6].sum()

    inter, card = sum_pt, sum_p + sum_t
    score = (2.0 * inter + SMOOTH) / max(card + SMOOTH, EPS)
    dice = (1.0 - score) * (1.0 if sum_t > 0 else 0.0)

    tprec = (sost + SMOOTH) / (so + SMOOTH)
    tsens = (sost + SMOOTH) / (st + SMOOTH)
    cl = 2.0 * tprec * tsens / (tprec + tsens)
    cld = (1.0 - cl) * (1.0 if st > 0 else 0.0)
    return np.float32((1.0 - ALPHA) * dice + ALPHA * cld)


def run_partials(output, target, trace=False):
    from concourse import bass_utils
    nc = get_nc()
    in_maps = []
    for c in range(N_CORES):
        in_maps.append({
            "logits": np.ascontiguousarray(output[c, 0], dtype=np.float32),
            "target": np.ascontiguousarray(target[c, 0], dtype=np.int32),
        })
    res = bass_utils.run_bass_kernel_spmd(
        nc, in_maps, core_ids=list(range(N_CORES)), trace=trace)
    parts = np.stack([res.results[c]["partials"] for c in range(N_CORES)])
    return parts, res


def kernel(output, target):
    parts, _ = run_partials(output, target)
    return combine_partials(parts)


# revision 21
# speedup vs baseline: 1.5496x; 1.5248x over previous
"""clDice loss kernel for Trainium2 (8 NeuronCores, batch-data-parallel).

Each core processes one [1024,1024] image pair (sigmoid + 10-iter soft
skeletonization + partial sums); the host combines per-core partial sums
in float64 and applies the dice/clDice formulas.

Layout: image row r lives at (partition p = r//8, block n = r%8), i.e.
an SBUF tile [128, 8, 1024].  3-point min/max pools are built from two
shifted tensor_tensor ops; vertical (cross-row) shifts that cross
partitions go through small SBUF->SBUF DMA halo copies with a
replicated-edge convention so every compute op spans all 128 partitions.
"""

import numpy as np

P, NB, W = 128, 8, 1024
N_ITER = 10
N_CORES = 8
SMOOTH = 1.0
EPS = 1e-7
ALPHA = 0.5

# engine knobs: 'v' = vector (DVE), 'g' = gpsimd
ENG_HPAIR_MIN = 'v'   # hpool pair op (erode path)
ENG_HCOMB_MIN = 'v'   # hpool combine op (erode path)
ENG_HPAIR_MAX = 'v'   # hpool pair op (dilate path)
ENG_HCOMB_MAX = 'v'   # hpool combine op (dilate path)
ENG_MULT = 'v'        # skel-update mul
ENG_ADD = 'g'         # skel-update add

_CACHE = {}


def _build_nc():
    import concourse.bacc as bacc
    import concourse.mybir as mybir
    import concourse.tile as tile

    DT = mybir.dt.bfloat16
    F32 = mybir.dt.float32
    I32 = mybir.dt.int32
    MIN = mybir.AluOpType.min
    MAX = mybir.AluOpType.max
    SUB = mybir.AluOpType.subtract
    ADD = mybir.AluOpType.add
    MULT = mybir.AluOpType.mult
    AF = mybir.ActivationFunctionType
    XY = mybir.AxisListType.XY

    nc = bacc.Bacc("TRN2", target_bir_lowering=False, debug=False,
                   num_devices=N_CORES)
    lg = nc.dram_tensor("logits", [P * NB, W], F32, kind="ExternalInput")
    tg = nc.dram_tensor("target", [P * NB, W], I32, kind="ExternalInput")
    sh = nc.dram_tensor("shifts", [4 * P, P], DT, kind="ExternalInput")
    pr = nc.dram_tensor("partials", [P, 8], F32, kind="ExternalOutput")

    lgv = lg.ap().rearrange("(p n) m -> p n m", n=NB)
    shv = sh.ap().rearrange("(i k) m -> k i m", i=4)
    tgv = tg.ap().rearrange("(p n) m -> p n m", n=NB)

    with tile.TileContext(nc) as tc:
        from contextlib import ExitStack
        ctx = ExitStack()
        with ctx:
            im_pool = ctx.enter_context(tc.tile_pool(name="im", bufs=2))
            sk_pool = ctx.enter_context(tc.tile_pool(name="sk", bufs=3))
            s_pool = ctx.enter_context(tc.tile_pool(name="s", bufs=4))
            halo_pool = ctx.enter_context(tc.tile_pool(name="halo", bufs=6))
            stage_pool = ctx.enter_context(tc.tile_pool(name="stage", bufs=2))
            p_pool = ctx.enter_context(tc.tile_pool(name="part", bufs=1))
            c_pool = ctx.enter_context(tc.tile_pool(name="const", bufs=1))
            ps_pool = ctx.enter_context(
                tc.tile_pool(name="ps", bufs=3, space="PSUM"))

            partials = p_pool.tile([P, 8], F32, tag="pp", name="pp")
            nc.vector.memset(partials[:, :], 0.0)
            shm = c_pool.tile([P, 4, P], DT, tag="shm", name="shm")
            nc.sync.dma_start(shm[:, :, :], shv[:, :, :])

            def eng(which):
                return nc.gpsimd if which == 'g' else nc.vector

            def img_tile(pool, tag):
                return pool.tile([P, NB, W], DT, tag=tag, name=tag)

            def pool3_h(dst, src, op, e_pair='v', e_comb='v', split_n0=False):
                # horizontal 3-pool along the W axis (shrink-at-edge)
                b = img_tile(s_pool, "s")
                eng(e_pair).tensor_tensor(
                    b[:, :, 0:W - 1], src[:, :, 0:W - 1], src[:, :, 1:W], op=op)
                nc.scalar.copy(dst[:, :, 0:1], b[:, :, 0:1])
                nc.scalar.copy(dst[:, :, W - 1:W], b[:, :, W - 2:W - 1])
                if split_n0:
                    # emit block-0 rows first so a following vertical pool
                    # can start its halo DMA early
                    eng(e_comb).tensor_tensor(
                        dst[:, 0, 1:W - 1], b[:, 0, 0:W - 2], src[:, 0, 2:W], op=op)
                    eng(e_comb).tensor_tensor(
                        dst[:, NB - 1, 1:W - 1], b[:, NB - 1, 0:W - 2],
                        src[:, NB - 1, 2:W], op=op)
                    eng(e_comb).tensor_tensor(
                        dst[:, 1:NB - 1, 1:W - 1], b[:, 1:NB - 1, 0:W - 2],
                        src[:, 1:NB - 1, 2:W], op=op)
                else:
                    eng(e_comb).tensor_tensor(
                        dst[:, :, 1:W - 1], b[:, :, 0:W - 2], src[:, :, 2:W], op=op)

            def pe_shift(dst, src, main_blk, fix_blk, mi, fi):
                """dst[p,:] = src[p +/- 1, main_blk, :] via shift-matrix matmul
                on the (idle) TensorEngine, with the boundary lane patched from
                fix_blk by a one-hot accumulate; ACT copies PSUM->SBUF."""
                ps = ps_pool.tile([P, W], F32, tag="ps", name="ps")
                for h in range(2):
                    sl = slice(512 * h, 512 * h + 512)
                    nc.tensor.matmul(ps[:, sl], shm[:, mi, :],
                                     src[:, main_blk, sl], start=True, stop=False)
                    nc.tensor.matmul(ps[:, sl], shm[:, fi, :],
                                     src[:, fix_blk, sl], start=False, stop=True)
                nc.scalar.copy(dst[:, :], ps[:, :])

            def pool3_v(dst, src, op):
                # vertical 3-pool across rows r = 8p+n (shrink-at-edge).
                # Cross-partition neighbor rows come from TensorEngine shift
                # matmuls (PSUM) -> ACT copy, keeping DVE and DMA queues free.
                a = img_tile(s_pool, "s")
                xd = halo_pool.tile([P, W], DT, tag="h", name="xd")
                # xd[p] = row below partition p's last row (bottom: replicate
                # of the image's last row, making the shrink-window exact)
                pe_shift(xd, src, 0, NB - 1, 0, 1)
                xu = halo_pool.tile([P, W], DT, tag="h", name="xu")
                # xu[p] = row above partition p's first row (top: row 0 itself
                # so the pair below stays exact)
                pe_shift(xu, src, NB - 1, 0, 2, 3)
                nc.vector.tensor_tensor(
                    a[:, 0:NB - 1, :], src[:, 0:NB - 1, :], src[:, 1:NB, :], op=op)
                au = halo_pool.tile([P, W], DT, tag="h", name="au")
                # au[p] = pair of rows (8p-1, 8p); at top: row 0 itself
                nc.vector.tensor_tensor(
                    au[:, :], xu[:, :], src[:, 0, :], op=op)
                nc.vector.tensor_tensor(
                    dst[:, 1:NB - 1, :], a[:, 0:NB - 2, :], src[:, 2:NB, :], op=op)
                nc.vector.tensor_tensor(
                    dst[:, NB - 1, :], a[:, NB - 2, :], xd[:, :], op=op)
                nc.vector.tensor_tensor(
                    dst[:, 0, :], au[:, :], src[:, 1, :], op=op)

            def erode(dst, src):
                vmin = img_tile(s_pool, "s")
                pool3_v(vmin, src, MIN)
                hmin = img_tile(s_pool, "s")
                pool3_h(hmin, src, MIN, ENG_HPAIR_MIN, ENG_HCOMB_MIN)
                # split the combine so the consumer vpool's halo DMAs (which
                # need blocks 0 and 7 of dst) can start before the bulk is done
                nc.vector.tensor_tensor(
                    dst[:, 0, :], vmin[:, 0, :], hmin[:, 0, :], op=MIN)
                nc.vector.tensor_tensor(
                    dst[:, NB - 1, :], vmin[:, NB - 1, :], hmin[:, NB - 1, :], op=MIN)
                nc.vector.tensor_tensor(
                    dst[:, 1:NB - 1, :], vmin[:, 1:NB - 1, :],
                    hmin[:, 1:NB - 1, :], op=MIN)

            def dilate(dst, src):
                hm = img_tile(s_pool, "s")
                pool3_h(hm, src, MAX, ENG_HPAIR_MAX, ENG_HCOMB_MAX, split_n0=True)
                pool3_v(dst, hm, MAX)

            def stream(im0, accum_col):
                # soft_skel with one erode per iteration; returns final skel
                e = img_tile(im_pool, "im")
                erode(e, im0)
                opn = img_tile(s_pool, "s")
                dilate(opn, e)
                t1 = img_tile(s_pool, "s")
                nc.vector.tensor_tensor(t1[:, :, :], im0[:, :, :], opn[:, :, :], op=SUB)
                sk = img_tile(sk_pool, "sk")
                nc.scalar.activation(sk[:, :, :], t1[:, :, :], AF.Relu)
                im = e
                for j in range(N_ITER):
                    # r depends only on last iteration's sk — emit first so
                    # ACT computes it during the erode/dilate phase
                    r = img_tile(s_pool, "s")
                    nc.scalar.activation(r[:, :, :], sk[:, :, :], AF.Relu,
                                         bias=1.0, scale=-1.0)
                    e = img_tile(im_pool, "im")
                    erode(e, im)
                    opn = img_tile(s_pool, "s")
                    dilate(opn, e)
                    t1 = img_tile(s_pool, "s")
                    nc.vector.tensor_tensor(
                        t1[:, :, :], im[:, :, :], opn[:, :, :], op=SUB)
                    # relu(t1)*r == relu(t1*r) since r >= 0: skip the ACT relu
                    # and clamp with a cheap 4x-mode tensor_scalar instead
                    q = img_tile(s_pool, "s")
                    nc.vector.tensor_tensor(
                        q[:, :, :], t1[:, :, :], r[:, :, :], op=MULT)
                    prod = img_tile(s_pool, "s")
                    nc.vector.tensor_scalar_max(prod[:, :, :], q[:, :, :], 0.0)
                    sk_new = img_tile(sk_pool, "sk")
                    if j < N_ITER - 1:
                        nc.vector.tensor_tensor(
                            sk_new[:, :, :], sk[:, :, :], prod[:, :, :], op=ADD)
                    else:
                        nc.vector.scalar_tensor_tensor(
                            sk_new[:, :, :], prod[:, :, :], 0.0, sk[:, :, :],
                            op0=ADD, op1=ADD,
                            accum_out=partials[:, accum_col:accum_col + 1])
                    sk = sk_new
                    im = e
                return sk

            # --- init: probs (sigmoid) + dice partial sums ---
            p0 = img_tile(im_pool, "im")
            for h in range(2):
                st = stage_pool.tile([P, NB // 2, W], F32, tag="st", name="st")
                nc.gpsimd.dma_start(st[:, :, :], lgv[:, 4 * h:4 * h + 4, :])
                nc.scalar.activation(p0[:, 4 * h:4 * h + 4, :], st[:, :, :],
                                     AF.Sigmoid,
                                     accum_out=partials[:, h:h + 1])
            tb = img_tile(s_pool, "s")
            for h in range(2):
                st = stage_pool.tile([P, NB // 2, W], I32, tag="st", name="st")
                nc.gpsimd.dma_start(st[:, :, :], tgv[:, 4 * h:4 * h + 4, :])
                nc.vector.tensor_copy(tb[:, 4 * h:4 * h + 4, :], st[:, :, :])
            nc.vector.tensor_reduce(partials[:, 2:3], tb[:, :, :], axis=XY, op=ADD)
            pt_scr = img_tile(s_pool, "s")
            nc.vector.scalar_tensor_tensor(
                pt_scr[:, :, :], p0[:, :, :], 0.0, tb[:, :, :],
                op0=ADD, op1=MULT, accum_out=partials[:, 3:4])

            # prefetch the target reload into stage tiles; the DMAs fire as
            # soon as the init casts release the slots, long before needed
            re_st = []
            for h in range(2):
                st = stage_pool.tile([P, NB // 2, W], I32, tag="st", name="st")
                nc.gpsimd.dma_start(st[:, :, :], tgv[:, 4 * h:4 * h + 4, :])
                re_st.append(st)

            # --- probs stream ---
            sk_o = stream(p0, accum_col=4)

            # --- target stream (cast prefetched reload) ---
            t0 = img_tile(im_pool, "im")
            for h in range(2):
                nc.vector.tensor_copy(t0[:, 4 * h:4 * h + 4, :], re_st[h][:, :, :])
            sk_t = stream(t0, accum_col=5)

            sost_scr = img_tile(s_pool, "s")
            nc.vector.scalar_tensor_tensor(
                sost_scr[:, :, :], sk_o[:, :, :], 0.0, sk_t[:, :, :],
                op0=ADD, op1=MULT, accum_out=partials[:, 6:7])

            nc.sync.dma_start(pr.ap(), partials[:, :])

    nc.compile()
    return nc


def get_nc():
    if "nc" not in _CACHE:
        _CACHE["nc"] = _build_nc()
    return _CACHE["nc"]


def combine_partials(parts):
    """parts: [n_cores, 128, 8] float array -> scalar loss (float32)."""
    parts = np.asarray(parts, dtype=np.float64)
    sum_p = parts[:, :, 0].sum() + parts[:, :, 1].sum()
    sum_t = parts[:, :, 2].sum()
    sum_pt = parts[:, :, 3].sum()
    so = parts[:, :, 4].sum()
    st = parts[:, :, 5].sum()
    sost = parts[:, :, 6].sum()

    inter, card = sum_pt, sum_p + sum_t
    score = (2.0 * inter + SMOOTH) / max(card + SMOOTH, EPS)
    dice = (1.0 - score) * (1.0 if sum_t > 0 else 0.0)

    tprec = (sost + SMOOTH) / (so + SMOOTH)
    tsens = (sost + SMOOTH) / (st + SMOOTH)
    cl = 2.0 * tprec * tsens / (tprec + tsens)
    cld = (1.0 - cl) * (1.0 if st > 0 else 0.0)
    return np.float32((1.0 - ALPHA) * dice + ALPHA * cld)


def _shift_mats():
    import ml_dtypes
    s_down = np.eye(P, k=-1)
    e_bot = np.zeros((P, P)); e_bot[P - 1, P - 1] = 1.0
    s_up = np.eye(P, k=1)
    e_top = np.zeros((P, P)); e_top[0, 0] = 1.0
    m = np.concatenate([s_down, e_bot, s_up, e_top], axis=0)
    return m.astype(ml_dtypes.bfloat16)


def run_partials(output, target, trace=False):
    from concourse import bass_utils
    nc = get_nc()
    sh = _shift_mats()
    in_maps = []
    for c in range(N_CORES):
        in_maps.append({
            "logits": np.ascontiguousarray(output[c, 0], dtype=np.float32),
            "target": np.ascontiguousarray(target[c, 0], dtype=np.int32),
            "shifts": sh,
        })
    res = bass_utils.run_bass_kernel_spmd(
        nc, in_maps, core_ids=list(range(N_CORES)), trace=trace)
    parts = np.stack([res.results[c]["partials"] for c in range(N_CORES)])
    return parts, res


def kernel(output, target):
    parts, _ = run_partials(output, target)
    return combine_partials(parts)
